# revision 24
# baseline (speedup 1.0000x reference)
"""Fused Conv3x3 + BatchNorm(train) + ReLU on 8 TRN2 NeuronCores.

Data-parallel over batch: each core processes 8 of the 64 images.
Conv is computed as matmuls over PSUM tiles of [128 out_ch, 512 pixels].
The 9 filter taps are covered by 6 matmuls per tile: 3 K=128 pairs
(taps (0,kw)+(1,kw)) plus 3 K=64 singles (taps (2,kw)). A single input
layout per image feeds all taps:
  xa: partitions 0-63 padded image, 64-127 same image shifted down one
      padded row
The K=64 singles run on the lower (rows 0-63) PE array half for even
tiles and the upper half (rows 64-127, reading the row-shifted copy
with the window moved up one row) for odd tiles, so adjacent tiles'
singles overlap in the array. The half is fixed per PSUM tile: the
hardware mishandles accumulation groups whose tile_position changes
more than once, so all three singles of a tile share one half.
Singles are issued first so K only grows within an accumulation
group.

Weight loads are amortized: each tap's LDWEIGHTS is followed by one
matmul per PSUM bank; redundant LDWEIGHTS that the tile legalizer
emits per-matmul are deleted post-compile (_dedupe_ldweights).

BatchNorm batch statistics: per-channel sum rides the PSUM->SBUF copy
(scalar activation accum), sum-of-squares is one DVE pass straight
from SBUF. The 1KB cross-core AllReduce is split in two: images 0-5
fold and reduce while the last two images' conv still runs, and only
the small remainder collective sits on the post-conv critical path
(the CC meshes serialize on the CC cores, so the second mesh starts
right after the first). A dummy AllReduce at kernel start absorbs the
~60us one-time CC bring-up, and a burst of zero matmuls releases the
PE HAM clock throttle (1.2 -> 2.4 GHz) before the first real matmul.
Scale/shift + ReLU are applied in chunks split ~3:2 between the
vector engine (affine tensor_scalar + max-with-0, 4x-packed bf16) and
the scalar engine (one fused Relu activation), with two output DMAs
per chunk on the sync ring draining bf16 results. Output is upcast to
fp32 on the host (BN output scale is O(1); bf16 rounding is ~0.4% per
element vs the 2e-2 gate).
"""

import numpy as np

import concourse.bacc as bacc
import concourse.tile as tile
from concourse import mybir
from concourse.bass_utils import run_bass_kernel_spmd

N_CORES = 8
IMG_PER_CORE = 8          # 64 images / 8 cores
C_IN = 64
C_OUT = 128
H = W = 64
HP, WP = H + 2, W + 2     # zero-padded image
PIX = H * W               # 4096
TILE_PX = 512             # one PSUM bank of fp32
ROWS_PER_TILE = TILE_PX // W       # 8
TILES_PER_IMG = PIX // TILE_PX     # 8
BN_EPS = 1e-5
COUNT = 64 * H * W        # batch-stat count over (N, H, W)

F32 = mybir.dt.float32
BF16 = mybir.dt.bfloat16

# Set by test harness to capture a profile; LAST_EXEC_NS holds the result.
KERNEL_TRACE = False
LAST_EXEC_NS = None

_cached_nc = None
import os as _os
OUT_BF16 = _os.environ.get("OUT_BF16", "1") == "1"
DVE_APPLY = _os.environ.get("DVE_APPLY", "1") == "1"
WARM = _os.environ.get("WARM", "1") == "1"

# conv groups: (img, first_tile, n_tiles). First image leads with 1-tile
# groups so compute starts after a small partial DMA; the last image ends
# with small groups so the post-conv stats tail is short.
GROUPS = [(0, 0, 1), (0, 1, 1), (0, 2, 2), (0, 4, 4)]
GROUPS += [(i, t, 4) for i in range(1, 7) for t in (0, 4)]
GROUPS += [(7, 0, 2), (7, 2, 2), (7, 4, 2), (7, 6, 1), (7, 7, 1)]
NG = len(GROUPS)          # 21
NG_CC1 = 14               # groups [0,14) = images 0-5 feed the first CC

# apply-phase chunks: (first_tile_global, n_tiles); small leading chunks
# prime the output-DMA pipe sooner, small trailing chunks shorten the
# final DMA drain. Chunks alternate vector/scalar engines.
CHUNKS = [(0, 1), (1, 1), (2, 2), (4, 4)]
CHUNKS += [(8 * i + t, 4) for i in range(1, 8) for t in (0, 4)]
CHUNKS = CHUNKS[:-1] + [(60, 2), (62, 1), (63, 1)]


def _dedupe_ldweights(nc):
    """Delete redundant InstLdweights the legalizer emits per-matmul.

    Consecutive matmuls that reuse the PE-resident weights keep only the
    first load. Only sync-free duplicates whose key (tensor, offset,
    pattern, dtype, PE tiling) matches the previous load are removed.
    """
    removed = 0
    for f in nc.m.functions:
        for blk in f.blocks:
            insts = blk.instructions
            keep = []
            last_key = None
            for i in insts:
                tn = type(i).__name__
                if tn == 'InstLdweights':
                    a = i.ins[0]
                    key = (a.memref, a.offset, str(a.ap), str(a.dtype),
                           i.tile_position, i.tile_size,
                           str(i.perf_mode), i.is_transpose)
                    si = i.sync_info
                    clean = si is None or (not si.on_wait and not si.on_update)
                    if clean and key == last_key:
                        removed += 1
                        continue
                    last_key = key
                elif tn != 'InstMatmult':
                    last_key = None
                keep.append(i)
            if removed and len(keep) != len(insts):
                del insts[:]
                for i in keep:
                    insts.append(i)
    return removed


def _build():
    nc = bacc.Bacc("TRN2", target_bir_lowering=False, debug=False,
                   num_devices=N_CORES)

    # Clear kernel-range semaphores at entry. The target_bir_lowering=False
    # path skips the per-kernel sem_clear, so stale semaphore values left by
    # a previous (crashed or foreign) kernel on the shared device would
    # satisfy this kernel's >=N waits early and corrupt results.
    from concourse.bass import compact_to_ranges
    for sem_range in compact_to_ranges(
            [s for s in nc._kernel_sem_range if s not in nc.barrier_sems]):
        nc.gpsimd.dma_reset(sem_range)
        nc.gpsimd.sem_clear(sem_range)
    nc._nrt_pseudo_barrier()

    xa_in = nc.dram_tensor("xa", [IMG_PER_CORE, 128, HP * WP], BF16,
                           kind="ExternalInput")
    wt_in = nc.dram_tensor("wt", [128, 6, 128], BF16, kind="ExternalInput")
    gb_in = nc.dram_tensor("gb", [128, 2], F32, kind="ExternalInput")
    out_d = nc.dram_tensor("out", [IMG_PER_CORE, C_OUT, PIX],
                           BF16 if OUT_BF16 else F32,
                           kind="ExternalOutput")
    import os
    # the remote-DMA stats exchange (USE_CC=0) crashes under NTFF
    # profiling on this runtime, so default to the collective AllReduce
    use_cc = os.environ.get("USE_CC", "1") == "1"
    split_cc = os.environ.get("SPLIT_CC", "1") == "1"
    warm_cc = os.environ.get("WARM_CC", "1") == "1"
    if use_cc:
        cc_in = nc.dram_tensor("cc_in", [128, 2], F32)
        cc_out = nc.dram_tensor("cc_out", [128, 2], F32, addr_space="Shared")
        if split_cc:
            cc_in1 = nc.dram_tensor("cc_in1", [128, 2], F32)
            cc_out1 = nc.dram_tensor("cc_out1", [128, 2], F32,
                                     addr_space="Shared")
        if warm_cc:
            cc_w_in = nc.dram_tensor("cc_w_in", [128, 2], F32)
            cc_w_out = nc.dram_tensor("cc_w_out", [128, 2], F32,
                                      addr_space="Shared")
        rsem = None
    else:
        # cross-core stats exchange via direct SBUF remote DMA
        rsem = nc.alloc_semaphore("bn_rsem")
        lsem = nc.alloc_semaphore("bn_lsem")

    with tile.TileContext(nc) as tc:
        with (
            tc.tile_pool(name="consts", bufs=1) as consts,
            tc.tile_pool(name="xa", bufs=2) as xa_pool,
            tc.tile_pool(name="ybuf", bufs=1) as ybuf_pool,
            tc.tile_pool(name="sq", bufs=2) as sq_pool,
            tc.tile_pool(name="zbuf", bufs=3) as z_pool,
            tc.tile_pool(name="stats", bufs=1) as stats_pool,
            tc.tile_pool(name="outp", bufs=16) as out_pool,
            tc.tile_pool(name="psum", bufs=2, space="PSUM") as psum_pool,
        ):
            wt = consts.tile([128, 6, 128], BF16)
            nc.sync.dma_start(out=wt[:], in_=wt_in[:])
            gb = consts.tile([128, 2], F32)
            nc.sync.dma_start(out=gb[:], in_=gb_in[:])
            eps_t = consts.tile([128, 1], F32)
            nc.vector.memset(eps_t[:], BN_EPS)
            if WARM:
                # prime the scalar engine's Sqrt and Relu tables while it
                # is idle so no ACT_TABLE_LOAD lands post-collective
                warm = consts.tile([128, 1], F32)
                nc.scalar.activation(warm[:], eps_t[:],
                                     mybir.ActivationFunctionType.Sqrt)
                nc.scalar.activation(warm[:], eps_t[:],
                                     mybir.ActivationFunctionType.Relu)

            # y stays resident in SBUF (bf16) between conv and BN apply.
            ybuf = ybuf_pool.tile([128, 64, TILE_PX], BF16)
            sums = stats_pool.tile([128, NG], F32)
            sumsqs = stats_pool.tile([128, NG], F32)

            if use_cc and warm_cc:
                # dummy AllReduce at kernel start: warms the CC-core path
                # (descriptor staging) while the PE is still ramping, so
                # the data collectives trigger with less latency
                wst = stats_pool.tile([128, 2], F32)
                nc.vector.memset(wst[:], 0.0)
                nc.sync.dma_start(out=cc_w_in[:], in_=wst[:])
                nc.gpsimd.collective_compute(
                    "AllReduce", mybir.AluOpType.add,
                    ins=[cc_w_in[:]], outs=[cc_w_out[:]],
                    replica_groups=[list(range(N_CORES))],
                )

            if WARM:
                # burst of dummy matmuls on a zeroed tile: ~4us of PE
                # activity releases the HAM clock throttle (4/8 -> 8/8)
                # before the first real matmul, which otherwise runs its
                # first ~3.4us at 1.2 GHz
                wsrc = consts.tile([128, TILE_PX], BF16)
                nc.vector.memset(wsrc[:], 0.0)
                # reuse group 0's PSUM tile (PSUM is fully budgeted):
                # group 0's first real matmul has start=True, so the
                # garbage accumulated here is discarded
                wps = psum_pool.tile([128, 1, TILE_PX], F32, tag="ps")
                for _ in range(10):
                    nc.tensor.matmul(
                        wps[:, 0, :], lhsT=wsrc[0:64, 0:128],
                        rhs=wsrc[0:64, :], start=True, stop=True,
                        skip_group_check=True,
                    )

            sb1 = stats_pool.tile([128, 2], F32)

            xa_t = {}

            def load_image(img, split):
                xa = xa_pool.tile([128, HP, WP], BF16)
                if split:
                    # land rows in the order the 1-tile lead groups read
                    for r0, r1 in ((0, 10), (10, 18), (18, 34), (34, HP)):
                        nc.sync.dma_start(
                            out=xa[:, r0:r1, :].rearrange("p a b -> p (a b)"),
                            in_=xa_in[img, :, r0 * WP:r1 * WP])
                else:
                    nc.sync.dma_start(
                        out=xa[:, :, :].rearrange("p a b -> p (a b)"),
                        in_=xa_in[img])
                xa_t[img] = xa

            load_image(0, split=True)

            for g, (img, t0, nt) in enumerate(GROUPS):
                if t0 == 0 and img + 1 < IMG_PER_CORE:
                    load_image(img + 1, split=False)
                xa = xa_t[img]
                if g == 0 and WARM:
                    ps = wps        # reuse the warm-up PSUM bank
                else:
                    ps = psum_pool.tile([128, nt, TILE_PX], F32)
                for s in range(6):
                    for tp in range(nt):
                        h0 = (t0 + tp) * ROWS_PER_TILE
                        if s < 3:
                            # singles (2, kw), K=64: alternate PE array
                            # halves so consecutive singles overlap. The
                            # upper half reads the +1-row-shifted copy, so
                            # its row window shifts by -1.
                            kw = s
                            if (t0 + tp) % 2 == 0:
                                lhsT = wt[0:64, 3 + kw, :]
                                rhs = xa[0:64, h0 + 2:h0 + 10, kw:kw + W]
                            else:
                                lhsT = wt[64:, 3 + kw, :]
                                rhs = xa[64:, h0 + 1:h0 + 9, kw:kw + W]
                        else:       # pairs (0,kw)+(1,kw), K=128
                            kw = s - 3
                            lhsT = wt[:, kw, :]
                            rhs = xa[:, h0:h0 + 8, kw:kw + W]
                        nc.tensor.matmul(
                            ps[:, tp, :], lhsT=lhsT, rhs=rhs,
                            start=(s == 0), stop=(s == 5),
                            skip_group_check=True,
                        )
                gt = img * TILES_PER_IMG + t0
                # PSUM -> SBUF copy + per-channel sum (scalar engine)
                nc.scalar.activation(
                    ybuf[:, gt:gt + nt, :], ps[:],
                    mybir.ActivationFunctionType.Copy,
                    accum_out=sums[:, g:g + 1],
                )
                # per-channel sum of squares in one DVE pass:
                # sq = (y * 1) * y, accum_out = sum(sq)
                yb = ybuf[:, gt:gt + nt, :]
                sq = sq_pool.tile([128, nt, TILE_PX], BF16)
                nc.vector.scalar_tensor_tensor(
                    out=sq[:], in0=yb, scalar=1.0, in1=yb,
                    op0=mybir.AluOpType.mult, op1=mybir.AluOpType.mult,
                    accum_out=sumsqs[:, g:g + 1],
                )
                if use_cc and split_cc and g == NG_CC1 - 1:
                    # first-stage stats over groups [0, NG_CC1): fold and
                    # AllReduce while conv continues, on the gpsimd queue
                    st1 = stats_pool.tile([128, 2], F32)
                    nc.vector.reduce_sum(st1[:, 0:1], sums[:, 0:NG_CC1],
                                         axis=mybir.AxisListType.X)
                    nc.vector.reduce_sum(st1[:, 1:2], sumsqs[:, 0:NG_CC1],
                                         axis=mybir.AxisListType.X)
                    nc.sync.dma_start(out=cc_in1[:], in_=st1[:])
                    nc.gpsimd.collective_compute(
                        "AllReduce", mybir.AluOpType.add,
                        ins=[cc_in1[:]], outs=[cc_out1[:]],
                        replica_groups=[list(range(N_CORES))],
                    )
                    nc.sync.dma_start(out=sb1[:], in_=cc_out1[:])

            # fold per-group partials, reduce across the 8 cores
            lo = NG_CC1 if (use_cc and split_cc) else 0
            st = stats_pool.tile([128, 2], F32)
            nc.vector.reduce_sum(st[:, 0:1], sums[:, lo:NG],
                                 axis=mybir.AxisListType.X)
            nc.vector.reduce_sum(st[:, 1:2], sumsqs[:, lo:NG],
                                 axis=mybir.AxisListType.X)
            g = stats_pool.tile([128, 2], F32)
            if use_cc:
                nc.sync.dma_start(out=cc_in[:], in_=st[:])
                nc.gpsimd.collective_compute(
                    "AllReduce",
                    mybir.AluOpType.add,
                    ins=[cc_in[:]],
                    outs=[cc_out[:]],
                    replica_groups=[list(range(N_CORES))],
                )
                if split_cc:
                    g2 = stats_pool.tile([128, 2], F32)
                    nc.sync.dma_start(out=g2[:], in_=cc_out[:])
                    nc.vector.scalar_tensor_tensor(
                        out=g[:], in0=sb1[:], scalar=1.0, in1=g2[:],
                        op0=mybir.AluOpType.mult, op1=mybir.AluOpType.add)
                else:
                    nc.sync.dma_start(out=g[:], in_=cc_out[:])
            else:
                # Each core lands its [128,2] partial in slot j of peer
                # (own_tpb XOR j)'s gather tile; slot 0 is the local copy.
                # Slot identity is irrelevant — slots are summed. The
                # gpsimd slot-0 copy orders the trigger after the folds
                # (queue order); each arriving broadcast bumps rsem by 2,
                # so rsem >= 14 (injected post-compile on the first gather
                # fold) means all 7 peers' partials have landed.
                gsts = stats_pool.tile([128, N_CORES, 2], F32)
                nc.gpsimd.tensor_copy(gsts[:, 0, :], st[:])
                for j in range(1, N_CORES):
                    rd = [None] * 8
                    rd[j] = (0, j)
                    nc.gpsimd.remote_dma_broadcast(
                        out_ap=gsts[:, j, :], in_ap=st[:],
                        remote_sem=rsem, local_sem=lsem, rdests=rd)
                nc.gpsimd.trigger_dma(count=None)
                nc.vector.reduce_sum(g[:, 0:1], gsts[:, :, 0:1],
                                     axis=mybir.AxisListType.XY)
                nc.vector.reduce_sum(g[:, 1:2], gsts[:, :, 1:2],
                                     axis=mybir.AxisListType.XY)

            # scale = gamma * rsqrt(var + eps); shift = beta - scale * mean
            # computed via negated means to keep every op fused:
            #   mv = [-mean, -E[y^2]]; negvar = mean^2 - E[y^2]
            #   sd = sqrt(-negvar + eps); scl = gamma / sd
            #   shv = scl * (-mean) + beta
            mv = stats_pool.tile([128, 2], F32)
            negvar = stats_pool.tile([128, 1], F32)
            sd = stats_pool.tile([128, 1], F32)
            inv = stats_pool.tile([128, 1], F32)
            scl = stats_pool.tile([128, 1], F32)
            shv = stats_pool.tile([128, 1], F32)
            nc.vector.tensor_scalar_mul(mv[:], g[:], -1.0 / COUNT)
            nc.vector.scalar_tensor_tensor(
                out=negvar[:], in0=mv[:, 0:1], scalar=mv[:, 0:1],
                in1=mv[:, 1:2],
                op0=mybir.AluOpType.mult, op1=mybir.AluOpType.add)
            nc.scalar.activation(sd[:], negvar[:],
                                 mybir.ActivationFunctionType.Sqrt,
                                 bias=eps_t[:], scale=-1.0)
            nc.vector.reciprocal(inv[:], sd[:])
            nc.vector.tensor_mul(scl[:], gb[:, 0:1], inv[:])
            nc.vector.scalar_tensor_tensor(
                out=shv[:], in0=scl[:], scalar=mv[:, 0:1], in1=gb[:, 1:2],
                op0=mybir.AluOpType.mult, op1=mybir.AluOpType.add)

            # apply: out = relu(y * scale + shift), alternating scalar /
            # vector engines per chunk, overlapping output DMA across both
            # HWDGE rings (sync + scalar)
            for ci, (t0, nt) in enumerate(CHUNKS):
                img, tl = divmod(t0, TILES_PER_IMG)
                ot = out_pool.tile([128, nt, TILE_PX],
                                   BF16 if OUT_BF16 else F32)
                use_dve = DVE_APPLY and ci % 5 in (0, 2, 3)
                if use_dve:
                    # DVE: z = y*scl + shv, then out = max(z, 0); both
                    # bf16 unit-stride -> 4x packed mode
                    z = z_pool.tile([128, nt, TILE_PX], BF16)
                    nc.vector.tensor_scalar(
                        z[:], ybuf[:, t0:t0 + nt, :], scl[:], shv[:],
                        op0=mybir.AluOpType.mult, op1=mybir.AluOpType.add)
                    nc.vector.tensor_scalar(
                        ot[:], z[:], 0.0, None, op0=mybir.AluOpType.max)
                else:
                    nc.scalar.activation(
                        ot[:], ybuf[:, t0:t0 + nt, :],
                        mybir.ActivationFunctionType.Relu,
                        bias=shv[:], scale=scl[:],
                    )
                px0 = tl * TILE_PX
                # two dma_starts per chunk keep more transfers in flight
                # so the 16 DMA queues stay saturated through the drain
                if nt >= 2:
                    h = nt // 2
                    nc.sync.dma_start(
                        out=out_d[img, :, px0:px0 + h * TILE_PX],
                        in_=ot[:, 0:h, :],
                    )
                    nc.sync.dma_start(
                        out=out_d[img, :, px0 + h * TILE_PX:
                                  px0 + nt * TILE_PX],
                        in_=ot[:, h:nt, :],
                    )
                else:
                    nc.sync.dma_start(
                        out=out_d[img, :, px0:px0 + nt * TILE_PX],
                        in_=ot[:],
                    )

    nc.compile()
    if os.environ.get("NO_DEDUPE") != "1":
        _dedupe_ldweights(nc)
    if rsem is not None:
        _inject_gather_wait(nc, rsem)
    return nc


def _inject_gather_wait(nc, rsem):
    """Insert `wait rsem >= 14` before the first cross-core gather fold.

    The gather folds read remote-DMA'd slots the tile scheduler cannot
    model (a pre-compile wait deadlocks its simulation), so the wait is
    injected post-compile as an EVENT_SEMAPHORE on the DVE queue right
    before the first of the two XY reduce instructions (the last two
    InstTensorReduce in program order).
    """
    import concourse.bass as bass
    reduces = []
    for f in nc.m.functions:
        for blk in f.blocks:
            for idx, i in enumerate(blk.instructions):
                if type(i).__name__ == 'InstTensorReduce':
                    reduces.append((blk, idx, i))
    assert len(reduces) == 4, len(reduces)
    blk, idx, fold = reduces[2]
    ev = mybir.InstEventSemaphore(
        name=nc.get_next_instruction_name(), ins=[], outs=[])
    ev.engine = fold.engine
    bass.BassInstruction(ev).wait_op(rsem, 14, "sem-ge")
    nc.register_instruction(ev)
    blk.instructions.insert(idx, ev)


def _prep_weights(weight: np.ndarray) -> np.ndarray:
    # [p, q, mb, mb] block matrix -> truncated OIHW kernel [128, 64, 3, 3]
    p, q, mb, _ = weight.shape
    Wm = weight.transpose(0, 2, 1, 3).reshape(p * mb, q * mb)
    Wm = Wm[:C_OUT, :C_IN * 9].reshape(C_OUT, C_IN, 3, 3)
    wt = np.zeros((128, 6, 128), np.float32)
    for k in range(3):                        # pairs (0,k)+(1,k)
        wt[:64, k, :] = Wm[:, :, 0, k].T
        wt[64:, k, :] = Wm[:, :, 1, k].T
    for k in range(3):                        # singles (2,k), both halves
        wt[:64, 3 + k, :] = Wm[:, :, 2, k].T
        wt[64:, 3 + k, :] = Wm[:, :, 2, k].T
    import ml_dtypes
    return wt.astype(np.dtype(ml_dtypes.bfloat16))


def kernel(x, weight, gamma, beta):
    global _cached_nc, LAST_EXEC_NS
    x = np.asarray(x, np.float32)
    weight = np.asarray(weight, np.float32)
    gamma = np.asarray(gamma, np.float32)
    beta = np.asarray(beta, np.float32)

    if _cached_nc is None:
        _cached_nc = _build()
    nc = _cached_nc

    wt = _prep_weights(weight)
    gb = np.ascontiguousarray(np.stack([gamma, beta], axis=1))
    import ml_dtypes
    bf16 = np.dtype(ml_dtypes.bfloat16)
    pad = np.zeros((64, C_IN, HP, WP), np.float32)
    pad[:, :, 1:H + 1, 1:W + 1] = x
    flat = pad.reshape(64, C_IN, HP * WP).astype(bf16)
    xa = np.zeros((64, 128, HP * WP), bf16)
    xa[:, :C_IN, :] = flat
    xa[:, C_IN:, :HP * WP - WP] = flat[:, :, WP:]   # shift down one row
    in_maps = []
    for i in range(N_CORES):
        sl = slice(i * IMG_PER_CORE, (i + 1) * IMG_PER_CORE)
        in_maps.append({
            "xa": np.ascontiguousarray(xa[sl]),
            "wt": wt, "gb": gb,
        })

    res = run_bass_kernel_spmd(nc, in_maps, list(range(N_CORES)),
                               trace=KERNEL_TRACE)
    LAST_EXEC_NS = res.exec_time_ns

    out = np.concatenate(
        [np.asarray(res.results[i]["out"], dtype=np.float32)
         .reshape(IMG_PER_CORE, C_OUT, H, W)
         for i in range(N_CORES)], axis=0)
    return out
